# revision 37
# baseline (speedup 1.0000x reference)
"""Trainium2 Bass kernel for nn_Attention_61830349193262.

Math per batch b (S = T = 2048, D = 1024):
    scores[s,t] = <state[s,:], x[t,:]>            (masked rows s where src==0)
    p_attn      = softmax_s(scores)               -> [S,T]
    w[t,d]      = sum_s state[s,d] p_attn[s,t]    (rows t where src==0 -> -inf)
    attn        = softmax_t(w)                    -> [T,D]
    out[e,d]    = sum_t state[t,d] attn[t,e]      -> [D,D]

Sharding: data-parallel over batch, one batch per NeuronCore (8 cores).

Masked rows contribute EXACTLY zero everywhere: p_attn is 0 at masked s,
attn is 0 at masked t, and masked-t columns of p_attn never reach the
output. The host therefore gathers the kept rows (same index set for s
and t since T==S share the mask), pads to a multiple of 128 (SPP), and
the device kernel runs entirely on the compacted [SPP, D] tensors --
~69% of the dense FLOPs with identical math. The [D, D] output needs no
scatter.

Device pipeline (per core):
  - All matmul operands are fp16 (full PE rate on trn2); PSUM
    accumulation and softmax statistics are fp32.
  - Phase-1a softmax runs directly on the PSUM score slabs: per-slab
    negated reduce_max combined with min-ops, then per-slab Exp on the
    scalar engine reading PSUM. No mask is needed: pad columns score
    exactly 0 while every real row max is >= ~60, so exp(0 - max)
    underflows pad entries to exactly 0 in fp16.
  - Phase 2 masks the pad tail multiplicatively after exp (keep vector)
    and skips the explicit normalize: unnormalized exp(w - max) goes to
    the PE and 1/z is folded into the per-partition scale of the scalar
    engine's PSUM->SBUF Copy (out rows are e, and z is per-e).
  - PSUM->SBUF copies run on the scalar engine (activation Copy) to
    keep the vector engine off the critical path.
  - x arrives host-transposed (x_t) so the per-t-block stationary tiles
    are plain strided DMAs, not slow xbar transposes. Only the on-chip
    etr / a_tr transposes use the DMA xbar.
  - HWDGE DMA transfers execute strictly serially (each waits for the
    previous transfer's completion semaphore), so ALL of them are
    triggered from the sync queue in exact consumption order: stq slabs
    + first x blocks first; the state_sig chunks (first needed by 1b,
    two superblocks in) trickle behind. keep rides the gpsimd SWDGE
    path, which is off that chain.
  - Superblock order: the tail (short) t-superblock is processed FIRST,
    so the final 1b superblock is a full-width one whose matmuls hide
    the first phase-2 softmax chains. The first 1b is deferred TWO
    superblocks (the later ones one) so the trickled state_sig loads
    are resident before it runs.
  - PE warmup: dummy matmuls woven between the startup stalls keep the
    PE clock ramped (it needs ~3us of continuous execution to reach
    2.4 GHz; every gap resets it to 1.2 GHz).
"""

import os
import numpy as np

_PHASES = int(os.environ.get("K_PHASES", "9"))  # debug bisect: 0=setup,1=+1a,2=+1b,9=full
_WARM = int(os.environ.get("K_WARM", "1"))

B, S, D = 8, 2048, 1024
ND = D // 128       # 8 d-chunks

_CACHED = {}


def _geometry(spp):
    # shared host/device tiling geometry: superblock slabs of <=512,
    # tail-first processing order, preloaded-block list
    slabs = []
    off = 0
    while off < spp:
        sz = min(512, spp - off)
        slabs.append((off, sz))
        off += sz
    nsup = len(slabs)
    porder = [nsup - 1] + list(range(nsup - 1)) if nsup > 1 else [0]
    tborder = []
    for ts in porder:
        to, tsz = slabs[ts]
        tborder += list(range(to // 128, (to + tsz) // 128))
    npre = min(4, spp // 128)
    return slabs, porder, tborder, npre


def _build(spp):
    import concourse.bass as bass
    import concourse.mybir as mybir
    import concourse.tile as tile
    from concourse import bacc

    f32 = mybir.dt.float32
    f16 = mybir.dt.float16
    Alu = mybir.AluOpType
    Act = mybir.ActivationFunctionType
    Ax = mybir.AxisListType

    NCH = spp // 128                       # s/t chunks of 128
    # s (and t) superblock slabs of <=512 (PSUM bank = 512 fp32);
    # tail (short) superblock processed first so the last 1b is full-width
    slabs, porder, tborder, NPRE = _geometry(spp)
    NSUP = len(slabs)
    DEFER = 2 if NSUP > 2 else 1           # 1b deferral depth at the start
    NHALF = ND // 2

    nc = bacc.Bacc("TRN2", target_bir_lowering=False, debug=False, num_devices=8)

    state_d = nc.dram_tensor("state", [spp, D], f16, kind="ExternalInput").ap()
    x_t_d = nc.dram_tensor("x_t", [D, spp], f16, kind="ExternalInput").ap()
    keep_d = nc.dram_tensor("keep", [spp], f16, kind="ExternalInput").ap()
    # startup tensors host-packed CONTIGUOUSLY in their exact on-chip
    # layouts, so every startup transfer on the serial DMA chain runs at
    # max line size instead of 256B/1KB strided lines (~2x the wire rate):
    # the 4 preloaded x blocks and the 8 stq half-slabs.
    xp_d = nc.dram_tensor("xp", [NPRE, 128, ND, 128], f16, kind="ExternalInput").ap()
    stqp_d = [
        [
            nc.dram_tensor(
                f"stqp{q}{h}", [128, NHALF, sz], f16, kind="ExternalInput"
            ).ap()
            for h in range(2)
        ]
        for q, (_, sz) in enumerate(slabs)
    ]
    out_d = nc.dram_tensor("out", [D, D], f32, kind="ExternalOutput").ap()

    with tile.TileContext(nc) as tc:
        with (
            tc.tile_pool(name="persist", bufs=1) as persist,
            tc.tile_pool(name="stage", bufs=5) as stage,
            tc.tile_pool(name="etr", bufs=3) as etrp,
            tc.tile_pool(name="work", bufs=2) as work,
            tc.tile_pool(name="sms", bufs=3) as smsp,
            tc.tile_pool(name="small", bufs=3) as small,
            tc.tile_pool(name="stats", bufs=12) as stats,
            tc.tile_pool(name="osb", bufs=2) as osb,
            tc.tile_pool(name="ps_s", bufs=6, space="PSUM") as ps_s,
            tc.tile_pool(name="ps_w", bufs=2, space="PSUM") as ps_w,
        ):
            # keep vector rides the gpsimd SWDGE path (off the serial HWDGE
            # chain), so it lands early without costing chain time. It doubles
            # as the warmup-matmul operand.
            keep_bc = persist.tile([128, spp], f16)
            keep_b = bass.AP(
                tensor=keep_d.tensor,
                offset=keep_d.offset,
                ap=[[0, 128]] + list(keep_d.ap),
            )
            nc.gpsimd.dma_start(out=keep_bc[:], in_=keep_b)

            def warm(n):
                # dummy matmuls to hold the PE pstate through startup stalls
                if not _WARM:
                    return
                for i in range(n):
                    pd = ps_w.tile([128, 512], f32, tag="pw", name=f"warm{nc.next_id()}")
                    nc.tensor.matmul(
                        pd[:], keep_bc[:, 0:128], keep_bc[:, 0:512],
                        start=True, stop=True,
                    )

            # ---- persistent inputs, in exact consumption order ----
            # stateT slabs arrive pre-packed from the host:
            #   stq[q][p2, dc, s'] = state[qoff + s', 128*dc + p2]
            # Every stq slab split into two half-dc pieces so matmul groups
            # unblock after each ~0.5 MB of (serial) DMA.
            xt_t = x_t_d.rearrange("(dc p) t -> p dc t", p=128)

            def stage_x(tb):
                # x_tr[p2, dc, t'] = x[tb*128 + t', 128*dc + p2], a plain
                # strided DMA out of host-transposed x_t. Stays on the HWDGE
                # chain: the SWDGE path proved too slow for this strided
                # pattern (PE starved every other t-block).
                x_tr = stage.tile(
                    [128, ND, 128], f16, tag="x_tr", name=f"x_tr_{tb}"
                )
                nc.sync.dma_start(
                    out=x_tr[:], in_=xt_t[:, :, tb * 128 : (tb + 1) * 128]
                )
                return x_tr

            NH = NHALF
            stqh = [
                [
                    persist.tile([128, NH, sz], f16, name=f"stq{q}{h}")
                    for h in range(2)
                ]
                for q, (_, sz) in enumerate(slabs)
            ]

            def stage_x_pre(i, tb):
                # preloaded x block from the host-packed contiguous copy
                x_tr = stage.tile(
                    [128, ND, 128], f16, tag="x_tr", name=f"x_tr_{tb}"
                )
                nc.sync.dma_start(out=x_tr[:], in_=xp_d[i])
                return x_tr

            x_pre = {}
            tb0 = tborder[0]
            x_pre[tb0] = stage_x_pre(0, tb0)   # first x block first
            for q in range(NSUP):
                for h in range(2):
                    nc.sync.dma_start(out=stqh[q][h][:], in_=stqp_d[q][h])
                if q == 0:
                    for i, tb in enumerate(tborder[1:2], start=1):
                        x_pre[tb] = stage_x_pre(i, tb)
            for i, tb in enumerate(tborder[2:NPRE], start=2):
                x_pre[tb] = stage_x_pre(i, tb)

            def stq_rhs(q, dc):
                return stqh[q][dc // NH][:, dc % NH, :]

            # state in natural s-chunks (first needed by the first 1b, two
            # superblocks in): trickled behind the startup loads, spread over
            # the 2nd..8th processed t-blocks.
            state_sig = [
                persist.tile([128, D], f16, name=f"ssig{c}") for c in range(NCH)
            ]
            NTRICK = min(7, NCH)
            trick = [[] for _ in range(NTRICK + 1)]
            for c in range(NCH):
                trick[1 + min(c * NTRICK // NCH, NTRICK - 1)].append(c)

            # wT[d, t] split per d-chunk: wt[dc][pd, t] = w[128*dc + pd, t]
            wt = [persist.tile([128, spp], f16, name=f"wt{dc}") for dc in range(ND)]

            if _PHASES == 0:
                dummy = osb.tile([128, D], f32, tag="out_sb")
                nc.vector.tensor_copy(dummy[:, 0:16], state_sig[0][:, 0:16])
                nc.vector.tensor_copy(dummy[:, 16:32], stqh[0][0][:, 0, 0:16])
                nc.sync.dma_start(out=out_d[0:128, :], in_=dummy[:])

            def stt_combine(a, b, op, tag):
                o = stats.tile([128, 1], f32, tag=tag)
                nc.vector.scalar_tensor_tensor(
                    out=o[:], in0=a[:], scalar=0.0, in1=b[:], op0=Alu.add, op1=op
                )
                return o

            def p2_softmax(ec):
                # softmax over t of wT chunk ec (no PE). Normalization is
                # deferred: rz2 is applied per-partition on the phase-2 PSUM
                # output rows instead of rescaling the [128, spp] exp matrix.
                wrow = wt[ec][:]  # [128, spp] f16, e = 128*ec + p
                nmax2 = stats.tile([128, 1], f32, tag="nmax2", name=f"nm2_{ec}")
                nc.vector.reduce_max(nmax2[:], wrow, axis=Ax.X, negate=True)
                a_raw = work.tile([128, spp], f16, tag="e_raw", name=f"a_raw_{ec}")
                nc.scalar.activation(
                    a_raw[:], wrow, Act.Exp, bias=nmax2[:], scale=1.0
                )
                a_m = smsp.tile([128, spp], f16, tag="a_m", name=f"a_m_{ec}")
                z2 = stats.tile([128, 1], f32, tag="z2", name=f"z2_{ec}")
                nc.vector.scalar_tensor_tensor(
                    out=a_m[:],
                    in0=a_raw[:],
                    scalar=1.0,
                    in1=keep_bc[:],
                    op0=Alu.mult,
                    op1=Alu.mult,
                    accum_out=z2[:],
                )
                rz2 = stats.tile([128, 1], f32, tag="rz2", name=f"rz2_{ec}")
                nc.vector.reciprocal(rz2[:], z2[:])
                a_tr = small.tile([128, NCH, 128], f16, tag="a_tr", name=f"a_tr_{ec}")
                nc.sync.dma_start(out=a_tr[:], in_=a_m[:], transpose=True)
                return a_tr, rz2

            def p2_matmul(ec, a_tr, rz2):
                out_sb = osb.tile([128, D], f32, tag="out_sb", name=f"osb_{ec}")
                for dh in range(2):
                    po = ps_s.tile([128, 512], f32, tag="psq", name=f"po_{ec}_{dh}")
                    for c4 in range(NCH):
                        nc.tensor.matmul(
                            po[:],
                            a_tr[:, c4, :],
                            state_sig[c4][:, dh * 512 : (dh + 1) * 512],
                            start=(c4 == 0),
                            stop=(c4 == NCH - 1),
                        )
                    # PSUM->SBUF copy + softmax normalize in one scalar-engine
                    # op: out rows are e, scaled per-partition by rz2[e]
                    nc.scalar.activation(
                        out_sb[:, dh * 512 : (dh + 1) * 512],
                        po[:],
                        Act.Copy,
                        scale=rz2[:],
                    )
                    nc.sync.dma_start(
                        out=out_d[ec * 128 : (ec + 1) * 128, dh * 512 : (dh + 1) * 512],
                        in_=out_sb[:, dh * 512 : (dh + 1) * 512],
                    )

            a_trs = {}
            N_INTERLEAVE = 3  # phase-2 softmaxes woven into the last 1b loop

            def phase_1b(ts, etr, weave):
                # wT[d, t] += state[s, d]^T E^T[s, t] for this superblock
                to, tsz = slabs[ts]
                for dc in range(ND if _PHASES >= 2 else 0):
                    pw = ps_w.tile([128, tsz], f32, tag="pw", name=f"pw_{ts}_{dc}")
                    for c3 in range(NCH):
                        nc.tensor.matmul(
                            pw[:],
                            state_sig[c3][:, dc * 128 : (dc + 1) * 128],
                            etr[:, c3, :],
                            start=(c3 == 0),
                            stop=(c3 == NCH - 1),
                        )
                    nc.scalar.activation(
                        wt[dc][:, to : to + tsz], pw[:], Act.Copy
                    )
                    # Weave the first phase-2 softmax chains (DVE/ACT/sync
                    # only) into the tail of phase 1 so their latency hides
                    # under the remaining 1b matmuls.
                    if _PHASES >= 3 and weave and dc < N_INTERLEAVE:
                        a_trs[dc] = p2_softmax(dc)

            # ---- phase 1: scores softmax -> E, then wT = state^T @ E^T ----
            pending_1b = []
            pidx = 0
            for pi, ts in enumerate(porder if _PHASES >= 1 else []):
                to, tsz = slabs[ts]
                ntb = tsz // 128
                etr = etrp.tile([128, NCH, tsz], f16, tag=f"etr{tsz}", name=f"etr_{ts}")
                for tbl in range(ntb):
                    tb = to // 128 + tbl
                    x_tr = x_pre[tb] if tb in x_pre else stage_x(tb)
                    if pidx < len(trick):
                        for c in trick[pidx]:
                            nc.gpsimd.dma_start(
                                out=state_sig[c][:],
                                in_=state_d[c * 128 : (c + 1) * 128, :],
                            )
                    pidx += 1

                    # scoresT[t', s] in PSUM slabs of [128, <=512]
                    quarters = []
                    for q in range(NSUP):
                        qo, qs = slabs[q]
                        psq = ps_s.tile([128, qs], f32, tag="psq")
                        for dc in range(ND):
                            nc.tensor.matmul(
                                psq[:],
                                x_tr[:, dc, :],
                                stq_rhs(q, dc),
                                start=(dc == 0),
                                stop=(dc == ND - 1),
                            )
                            if pidx <= 2 and dc == NH - 1:
                                # hold the PE clock between the stq half-tiles
                                warm(3 if pidx == 1 else 2)
                        quarters.append(psq)
                        if pidx <= 2:
                            # hold the PE clock through the startup stalls
                            warm((6, 3)[pidx - 1] if q < NSUP - 1 else 2)

                    # Row softmax straight from PSUM. Pad columns hold score
                    # exactly 0; every real row max is >= ~60, so
                    # exp(0 - max) underflows pad entries to exactly 0.
                    nmq = []
                    for q in range(NSUP):
                        m = stats.tile([128, 1], f32, tag=f"pm{q}")
                        nc.vector.reduce_max(
                            m[:], quarters[q][:], axis=Ax.X, negate=True
                        )
                        nmq.append(m)
                    while len(nmq) > 1:
                        nxt = []
                        for i in range(0, len(nmq) - 1, 2):
                            nxt.append(
                                stt_combine(nmq[i], nmq[i + 1], Alu.min, "pmc")
                            )
                        if len(nmq) % 2:
                            nxt.append(nmq[-1])
                        nmq = nxt
                    nmax = nmq[0]

                    e_raw = work.tile([128, spp], f16, tag="e_raw")
                    zq = []
                    for q in range(NSUP):
                        qo, qs = slabs[q]
                        z = stats.tile([128, 1], f32, tag=f"zq{q}")
                        nc.scalar.activation(
                            e_raw[:, qo : qo + qs],
                            quarters[q][:],
                            Act.Exp,
                            bias=nmax[:],
                            scale=1.0,
                            accum_out=z[:],
                        )
                        zq.append(z)
                    while len(zq) > 1:
                        nxt = []
                        for i in range(0, len(zq) - 1, 2):
                            nxt.append(stt_combine(zq[i], zq[i + 1], Alu.add, "zc"))
                        if len(zq) % 2:
                            nxt.append(zq[-1])
                        zq = nxt
                    rz = stats.tile([128, 1], f32, tag="rz")
                    nc.vector.reciprocal(rz[:], zq[0][:])
                    # E^T into etr: etr[p3, c3, tbl*128 + t'] = e_n[t', 128*c3 + p3]
                    # Normalize + transpose in two halves so the LAST piece of
                    # the chain is short: the next 1b's matmuls wait (via
                    # semaphore aggregation) on the final transpose emitted
                    # before them, so its latency after the last matmul is
                    # exposed at every superblock boundary.
                    e_n = work.tile([128, spp], f16, tag="e_n")
                    hsplit = (NCH // 2) * 128
                    for h0, h1 in ((0, hsplit), (hsplit, spp)):
                        nc.vector.tensor_scalar_mul(
                            e_n[:, h0:h1], e_raw[:, h0:h1], rz[:]
                        )
                        nc.sync.dma_start(
                            out=etr[:, h0 // 128 : h1 // 128, tbl * 128 : (tbl + 1) * 128],
                            in_=e_n[:, h0:h1],
                            transpose=True,
                        )

                pending_1b.append((ts, etr))
                if pi >= DEFER:
                    phase_1b(*pending_1b.pop(0), weave=False)
            for i, args in enumerate(pending_1b):
                phase_1b(*args, weave=(i == len(pending_1b) - 1))

            # ---- phase 2: out = attn^T @ state per e-chunk ----
            for ec in range(ND if _PHASES >= 3 else 0):
                # depth-2 software pipeline: softmax chain for ec+1 is
                # emitted before the matmuls of ec
                if ec == 0 and ec not in a_trs:
                    a_trs[ec] = p2_softmax(ec)
                if ec + 1 < ND and (ec + 1) not in a_trs:
                    a_trs[ec + 1] = p2_softmax(ec + 1)
                a_tr, rz2 = a_trs.pop(ec)
                p2_matmul(ec, a_tr, rz2)

    nc.compile()
    return nc


def get_nc(spp):
    if spp not in _CACHED:
        _CACHED[spp] = _build(spp)
    return _CACHED[spp]


def _make_in_maps(state, x, src):
    # Host-side prep: gather kept rows (src != 0), cast to fp16, pad to a
    # multiple of 128, and pre-transpose both state and x. fp16 conversion
    # happens host-side: the device would round both operands to fp16 before
    # the matmuls anyway (same numerics), and this halves input DMA bytes.
    state = np.asarray(state, dtype=np.float16)
    x = np.asarray(x, dtype=np.float16)
    src = np.asarray(src)
    idxs = [np.flatnonzero(src[b] != 0) for b in range(B)]
    smax = max(len(i) for i in idxs)
    spp = max(128, ((smax + 127) // 128) * 128)
    slabs, porder, tborder, npre = _geometry(spp)
    nh = ND // 2
    in_maps = []
    for b in range(B):
        idx = idxs[b]
        n = len(idx)
        sg = np.zeros((spp, D), np.float16)
        sg[:n] = state[b][idx]
        xg = np.zeros((spp, D), np.float16)
        xg[:n] = x[b][idx]
        keep = np.zeros(spp, np.float16)
        keep[:n] = 1.0
        # [p, dc, s] layouts for the packed startup tensors
        st_t3 = np.ascontiguousarray(sg.T).reshape(ND, 128, spp).transpose(1, 0, 2)
        xt = np.ascontiguousarray(xg.T)
        xt3 = xt.reshape(ND, 128, spp).transpose(1, 0, 2)
        m = {
            "state": sg,
            "x_t": xt,
            "keep": keep,
            "xp": np.ascontiguousarray(
                np.stack(
                    [
                        xt3[:, :, tb * 128 : (tb + 1) * 128]
                        for tb in tborder[:npre]
                    ]
                )
            ),
        }
        for q, (qo, qs) in enumerate(slabs):
            for h in range(2):
                m[f"stqp{q}{h}"] = np.ascontiguousarray(
                    st_t3[:, h * nh : (h + 1) * nh, qo : qo + qs]
                )
        in_maps.append(m)
    return in_maps, spp


def run_bass(state, x, src, trace=False, **trace_kwargs):
    from concourse.bass_utils import run_bass_kernel_spmd

    in_maps, spp = _make_in_maps(state, x, src)
    nc = get_nc(spp)
    res = run_bass_kernel_spmd(
        nc, in_maps, core_ids=list(range(B)), trace=trace, **trace_kwargs
    )
    out = np.stack([res.results[b]["out"] for b in range(B)]).astype(np.float32)
    return out, res


def kernel(state, x, src, **kwargs):
    out, _ = run_bass(state, x, src, trace=False)
    return out


if __name__ == "__main__":
    rng = np.random.default_rng(0)
    st = rng.standard_normal((B, S, D), dtype=np.float32)
    xx = rng.standard_normal((B, S, D), dtype=np.float32)
    sr = rng.integers(0, 5, size=(B, S))
    o = kernel(state=st, x=xx, src=sr)
    print(o.shape, o.dtype, np.abs(o).max())


# revision 39
# speedup vs baseline: 1.1808x; 1.1808x over previous
"""Trainium2 Bass kernel for nn_Attention_61830349193262.

Math per batch b (S = T = 2048, D = 1024):
    scores[s,t] = <state[s,:], x[t,:]>            (masked rows s where src==0)
    p_attn      = softmax_s(scores)               -> [S,T]
    w[t,d]      = sum_s state[s,d] p_attn[s,t]    (rows t where src==0 -> -inf)
    attn        = softmax_t(w)                    -> [T,D]
    out[e,d]    = sum_t state[t,d] attn[t,e]      -> [D,D]

Sharding: data-parallel over batch, one batch per NeuronCore (8 cores).

Masked rows contribute EXACTLY zero everywhere: p_attn is 0 at masked s,
attn is 0 at masked t, and masked-t columns of p_attn never reach the
output. The host therefore gathers the kept rows (same index set for s
and t since T==S share the mask), pads to a multiple of 128 (SPP), and
the device kernel runs entirely on the compacted [SPP, D] tensors --
~69% of the dense FLOPs with identical math. The [D, D] output needs no
scatter.

Device pipeline (per core):
  - All matmul operands are fp16 (full PE rate on trn2); PSUM
    accumulation and softmax statistics are fp32.
  - Phase-1a softmax runs directly on the PSUM score slabs: per-slab
    negated reduce_max combined with min-ops, then per-slab Exp on the
    scalar engine reading PSUM. No mask is needed: pad columns score
    exactly 0 while every real row max is >= ~60, so exp(0 - max)
    underflows pad entries to exactly 0 in fp16.
  - Phase 2 masks the pad tail multiplicatively after exp (keep vector)
    and skips the explicit normalize: unnormalized exp(w - max) goes to
    the PE and 1/z is folded into the per-partition scale of the scalar
    engine's PSUM->SBUF Copy (out rows are e, and z is per-e).
  - PSUM->SBUF copies run on the scalar engine (activation Copy) to
    keep the vector engine off the critical path.
  - x arrives host-transposed (x_t) so the per-t-block stationary tiles
    are plain strided DMAs, not slow xbar transposes. Only the on-chip
    etr / a_tr transposes use the DMA xbar.
  - HWDGE DMA transfers execute strictly serially (each waits for the
    previous transfer's completion semaphore), so ALL of them are
    triggered from the sync queue in exact consumption order: stq slabs
    + first x blocks first; the state_sig chunks (first needed by 1b,
    two superblocks in) trickle behind. keep rides the gpsimd SWDGE
    path, which is off that chain.
  - Superblock order: the tail (short) t-superblock is processed FIRST,
    so the final 1b superblock is a full-width one whose matmuls hide
    the first phase-2 softmax chains. The first 1b is deferred TWO
    superblocks (the later ones one) so the trickled state_sig loads
    are resident before it runs.
  - PE warmup: dummy matmuls woven between the startup stalls keep the
    PE clock ramped (it needs ~3us of continuous execution to reach
    2.4 GHz; every gap resets it to 1.2 GHz).
"""

import os
import numpy as np

_PHASES = int(os.environ.get("K_PHASES", "9"))  # debug bisect: 0=setup,1=+1a,2=+1b,9=full
_WARM = int(os.environ.get("K_WARM", "1"))

B, S, D = 8, 2048, 1024
ND = D // 128       # 8 d-chunks

_CACHED = {}


def _build(spp):
    import concourse.bass as bass
    import concourse.mybir as mybir
    import concourse.tile as tile
    from concourse import bacc

    f32 = mybir.dt.float32
    f16 = mybir.dt.float16
    Alu = mybir.AluOpType
    Act = mybir.ActivationFunctionType
    Ax = mybir.AxisListType

    NCH = spp // 128                       # s/t chunks of 128
    # s (and t) superblock slabs of <=512 (PSUM bank = 512 fp32)
    slabs = []
    off = 0
    while off < spp:
        sz = min(512, spp - off)
        slabs.append((off, sz))
        off += sz
    NSUP = len(slabs)
    # process the tail (short) superblock first so the last 1b is full-width
    porder = [NSUP - 1] + list(range(NSUP - 1)) if NSUP > 1 else [0]
    tborder = []
    for ts in porder:
        to, tsz = slabs[ts]
        tborder += list(range(to // 128, (to + tsz) // 128))
    NPRE = min(4, NCH)                     # t-blocks with preloaded x_tr
    DEFER = 2 if NSUP > 2 else 1           # 1b deferral depth at the start

    nc = bacc.Bacc("TRN2", target_bir_lowering=False, debug=False, num_devices=8)

    state_d = nc.dram_tensor("state", [spp, D], f16, kind="ExternalInput").ap()
    state_t_d = nc.dram_tensor("state_t", [D, spp], f16, kind="ExternalInput").ap()
    x_t_d = nc.dram_tensor("x_t", [D, spp], f16, kind="ExternalInput").ap()
    keep_d = nc.dram_tensor("keep", [spp], f16, kind="ExternalInput").ap()
    out_d = nc.dram_tensor("out", [D, D], f32, kind="ExternalOutput").ap()

    with tile.TileContext(nc) as tc:
        with (
            tc.tile_pool(name="persist", bufs=1) as persist,
            # deep x_tr rotation: a pool-gated x load BLOCKS every transfer
            # behind it on the strictly-serial HWDGE chain (incl. the etr
            # transposes that gate 1b), so keep the gate far from the front
            tc.tile_pool(name="stage", bufs=10) as stage,
            tc.tile_pool(name="etr", bufs=3) as etrp,
            tc.tile_pool(name="work", bufs=2) as work,
            tc.tile_pool(name="sms", bufs=3) as smsp,
            tc.tile_pool(name="small", bufs=3) as small,
            tc.tile_pool(name="stats", bufs=12) as stats,
            tc.tile_pool(name="osb", bufs=2) as osb,
            tc.tile_pool(name="ps_s", bufs=6, space="PSUM") as ps_s,
            tc.tile_pool(name="ps_w", bufs=2, space="PSUM") as ps_w,
        ):
            # keep vector rides the gpsimd SWDGE path (off the serial HWDGE
            # chain), so it lands early without costing chain time. It doubles
            # as the warmup-matmul operand.
            keep_bc = persist.tile([128, spp], f16)
            keep_b = bass.AP(
                tensor=keep_d.tensor,
                offset=keep_d.offset,
                ap=[[0, 128]] + list(keep_d.ap),
            )
            nc.gpsimd.dma_start(out=keep_bc[:], in_=keep_b)

            def warm(n):
                # dummy matmuls to hold the PE pstate through startup stalls
                if not _WARM:
                    return
                for i in range(n):
                    pd = ps_w.tile([128, 512], f32, tag="pw", name=f"warm{nc.next_id()}")
                    nc.tensor.matmul(
                        pd[:], keep_bc[:, 0:128], keep_bc[:, 0:512],
                        start=True, stop=True,
                    )

            # ---- persistent inputs, in exact consumption order ----
            # stateT slabs from the host-transposed state_t:
            #   stq[q][p2, dc, s'] = state[qoff + s', 128*dc + p2]
            # Slab 0 is split into two half-dc tiles so the first matmul
            # group can start after ~0.5 MB of (serial) DMA.
            st_t = state_t_d.rearrange("(dc p) s -> p dc s", p=128)
            xt_t = x_t_d.rearrange("(dc p) t -> p dc t", p=128)

            def stage_x(tb):
                # x_tr[p2, dc, t'] = x[tb*128 + t', 128*dc + p2], a plain
                # strided DMA out of host-transposed x_t. Stays on the HWDGE
                # chain: the SWDGE path proved too slow for this strided
                # pattern (PE starved every other t-block).
                x_tr = stage.tile(
                    [128, ND, 128], f16, tag="x_tr", name=f"x_tr_{tb}"
                )
                nc.sync.dma_start(
                    out=x_tr[:], in_=xt_t[:, :, tb * 128 : (tb + 1) * 128]
                )
                return x_tr

            # every stq slab split into two half-dc tiles so matmul groups
            # unblock after each ~0.5 MB of (serial) DMA
            NH = ND // 2
            stqh = [
                [
                    persist.tile([128, NH, sz], f16, name=f"stq{q}{h}")
                    for h in range(2)
                ]
                for q, (_, sz) in enumerate(slabs)
            ]
            x_pre = {}
            tb0 = tborder[0]
            x_pre[tb0] = stage_x(tb0)          # first x block first
            for q in range(NSUP):
                qo, qs = slabs[q]
                for h in range(2):
                    nc.sync.dma_start(
                        out=stqh[q][h][:],
                        in_=st_t[:, h * NH : (h + 1) * NH, qo : qo + qs],
                    )
                if q == 0:
                    for tb in tborder[1:2]:
                        x_pre[tb] = stage_x(tb)
            for tb in tborder[2:NPRE]:
                x_pre[tb] = stage_x(tb)

            def stq_rhs(q, dc):
                return stqh[q][dc // NH][:, dc % NH, :]

            # state in natural s-chunks (first needed by the first 1b, two
            # superblocks in): trickled behind the startup loads, spread over
            # the 2nd..8th processed t-blocks.
            state_sig = [
                persist.tile([128, D], f16, name=f"ssig{c}") for c in range(NCH)
            ]
            NTRICK = min(7, NCH)
            trick = [[] for _ in range(NTRICK + 1)]
            for c in range(NCH):
                trick[1 + min(c * NTRICK // NCH, NTRICK - 1)].append(c)

            # wT[d, t] split per d-chunk: wt[dc][pd, t] = w[128*dc + pd, t]
            wt = [persist.tile([128, spp], f16, name=f"wt{dc}") for dc in range(ND)]

            if _PHASES == 0:
                dummy = osb.tile([128, D], f32, tag="out_sb")
                nc.vector.tensor_copy(dummy[:, 0:16], state_sig[0][:, 0:16])
                nc.vector.tensor_copy(dummy[:, 16:32], stqh[0][0][:, 0, 0:16])
                nc.sync.dma_start(out=out_d[0:128, :], in_=dummy[:])

            def stt_combine(a, b, op, tag):
                o = stats.tile([128, 1], f32, tag=tag)
                nc.vector.scalar_tensor_tensor(
                    out=o[:], in0=a[:], scalar=0.0, in1=b[:], op0=Alu.add, op1=op
                )
                return o

            def p2_softmax(ec):
                # softmax over t of wT chunk ec (no PE). Normalization is
                # deferred: rz2 is applied per-partition on the phase-2 PSUM
                # output rows instead of rescaling the [128, spp] exp matrix.
                wrow = wt[ec][:]  # [128, spp] f16, e = 128*ec + p
                nmax2 = stats.tile([128, 1], f32, tag="nmax2", name=f"nm2_{ec}")
                nc.vector.reduce_max(nmax2[:], wrow, axis=Ax.X, negate=True)
                a_raw = work.tile([128, spp], f16, tag="e_raw", name=f"a_raw_{ec}")
                nc.scalar.activation(
                    a_raw[:], wrow, Act.Exp, bias=nmax2[:], scale=1.0
                )
                a_m = smsp.tile([128, spp], f16, tag="a_m", name=f"a_m_{ec}")
                z2 = stats.tile([128, 1], f32, tag="z2", name=f"z2_{ec}")
                nc.vector.scalar_tensor_tensor(
                    out=a_m[:],
                    in0=a_raw[:],
                    scalar=1.0,
                    in1=keep_bc[:],
                    op0=Alu.mult,
                    op1=Alu.mult,
                    accum_out=z2[:],
                )
                rz2 = stats.tile([128, 1], f32, tag="rz2", name=f"rz2_{ec}")
                nc.vector.reciprocal(rz2[:], z2[:])
                a_tr = small.tile([128, NCH, 128], f16, tag="a_tr", name=f"a_tr_{ec}")
                nc.sync.dma_start(out=a_tr[:], in_=a_m[:], transpose=True)
                return a_tr, rz2

            def p2_matmul(ec, a_tr, rz2):
                out_sb = osb.tile([128, D], f32, tag="out_sb", name=f"osb_{ec}")
                for dh in range(2):
                    po = ps_s.tile([128, 512], f32, tag="psq", name=f"po_{ec}_{dh}")
                    for c4 in range(NCH):
                        nc.tensor.matmul(
                            po[:],
                            a_tr[:, c4, :],
                            state_sig[c4][:, dh * 512 : (dh + 1) * 512],
                            start=(c4 == 0),
                            stop=(c4 == NCH - 1),
                        )
                    # PSUM->SBUF copy + softmax normalize in one scalar-engine
                    # op: out rows are e, scaled per-partition by rz2[e]
                    nc.scalar.activation(
                        out_sb[:, dh * 512 : (dh + 1) * 512],
                        po[:],
                        Act.Copy,
                        scale=rz2[:],
                    )
                    nc.sync.dma_start(
                        out=out_d[ec * 128 : (ec + 1) * 128, dh * 512 : (dh + 1) * 512],
                        in_=out_sb[:, dh * 512 : (dh + 1) * 512],
                    )

            a_trs = {}
            N_INTERLEAVE = 3  # phase-2 softmaxes woven into the last 1b loop

            def phase_1b(ts, etr, weave):
                # wT[d, t] += state[s, d]^T E^T[s, t] for this superblock
                to, tsz = slabs[ts]
                for dc in range(ND if _PHASES >= 2 else 0):
                    pw = ps_w.tile([128, tsz], f32, tag="pw", name=f"pw_{ts}_{dc}")
                    for c3 in range(NCH):
                        nc.tensor.matmul(
                            pw[:],
                            state_sig[c3][:, dc * 128 : (dc + 1) * 128],
                            etr[:, c3, :],
                            start=(c3 == 0),
                            stop=(c3 == NCH - 1),
                        )
                    nc.scalar.activation(
                        wt[dc][:, to : to + tsz], pw[:], Act.Copy
                    )
                    # Weave the first phase-2 softmax chains (DVE/ACT/sync
                    # only) into the tail of phase 1 so their latency hides
                    # under the remaining 1b matmuls.
                    if _PHASES >= 3 and weave and dc < N_INTERLEAVE:
                        a_trs[dc] = p2_softmax(dc)

            # ---- phase 1: scores softmax -> E, then wT = state^T @ E^T ----
            pending_1b = []
            pidx = 0
            for pi, ts in enumerate(porder if _PHASES >= 1 else []):
                to, tsz = slabs[ts]
                ntb = tsz // 128
                etr = etrp.tile([128, NCH, tsz], f16, tag=f"etr{tsz}", name=f"etr_{ts}")
                for tbl in range(ntb):
                    tb = to // 128 + tbl
                    x_tr = x_pre[tb] if tb in x_pre else stage_x(tb)
                    if pidx < len(trick):
                        for c in trick[pidx]:
                            nc.gpsimd.dma_start(
                                out=state_sig[c][:],
                                in_=state_d[c * 128 : (c + 1) * 128, :],
                            )
                    pidx += 1

                    # scoresT[t', s] in PSUM slabs of [128, <=512]
                    quarters = []
                    for q in range(NSUP):
                        qo, qs = slabs[q]
                        psq = ps_s.tile([128, qs], f32, tag="psq")
                        for dc in range(ND):
                            nc.tensor.matmul(
                                psq[:],
                                x_tr[:, dc, :],
                                stq_rhs(q, dc),
                                start=(dc == 0),
                                stop=(dc == ND - 1),
                            )
                            if pidx <= 2 and dc == NH - 1:
                                # hold the PE clock between the stq half-tiles
                                warm(3 if pidx == 1 else 2)
                        quarters.append(psq)
                        if pidx <= 2:
                            # hold the PE clock through the startup stalls
                            warm((6, 3)[pidx - 1] if q < NSUP - 1 else 2)

                    # Row softmax straight from PSUM. Pad columns hold score
                    # exactly 0; every real row max is >= ~60, so
                    # exp(0 - max) underflows pad entries to exactly 0.
                    nmq = []
                    for q in range(NSUP):
                        m = stats.tile([128, 1], f32, tag=f"pm{q}")
                        nc.vector.reduce_max(
                            m[:], quarters[q][:], axis=Ax.X, negate=True
                        )
                        nmq.append(m)
                    while len(nmq) > 1:
                        nxt = []
                        for i in range(0, len(nmq) - 1, 2):
                            nxt.append(
                                stt_combine(nmq[i], nmq[i + 1], Alu.min, "pmc")
                            )
                        if len(nmq) % 2:
                            nxt.append(nmq[-1])
                        nmq = nxt
                    nmax = nmq[0]

                    e_raw = work.tile([128, spp], f16, tag="e_raw")
                    zq = []
                    for q in range(NSUP):
                        qo, qs = slabs[q]
                        z = stats.tile([128, 1], f32, tag=f"zq{q}")
                        nc.scalar.activation(
                            e_raw[:, qo : qo + qs],
                            quarters[q][:],
                            Act.Exp,
                            bias=nmax[:],
                            scale=1.0,
                            accum_out=z[:],
                        )
                        zq.append(z)
                    while len(zq) > 1:
                        nxt = []
                        for i in range(0, len(zq) - 1, 2):
                            nxt.append(stt_combine(zq[i], zq[i + 1], Alu.add, "zc"))
                        if len(zq) % 2:
                            nxt.append(zq[-1])
                        zq = nxt
                    rz = stats.tile([128, 1], f32, tag="rz")
                    nc.vector.reciprocal(rz[:], zq[0][:])
                    # E^T into etr: etr[p3, c3, tbl*128 + t'] = e_n[t', 128*c3 + p3]
                    # Normalize + transpose in two halves so the LAST piece of
                    # the chain is short: the next 1b's matmuls wait (via
                    # semaphore aggregation) on the final transpose emitted
                    # before them, so its latency after the last matmul is
                    # exposed at every superblock boundary.
                    e_n = work.tile([128, spp], f16, tag="e_n")
                    hsplit = (NCH // 2) * 128
                    for h0, h1 in ((0, hsplit), (hsplit, spp)):
                        nc.vector.tensor_scalar_mul(
                            e_n[:, h0:h1], e_raw[:, h0:h1], rz[:]
                        )
                        nc.sync.dma_start(
                            out=etr[:, h0 // 128 : h1 // 128, tbl * 128 : (tbl + 1) * 128],
                            in_=e_n[:, h0:h1],
                            transpose=True,
                        )

                pending_1b.append((ts, etr))
                if pi >= DEFER:
                    phase_1b(*pending_1b.pop(0), weave=False)
            for i, args in enumerate(pending_1b):
                phase_1b(*args, weave=(i == len(pending_1b) - 1))

            # ---- phase 2: out = attn^T @ state per e-chunk ----
            for ec in range(ND if _PHASES >= 3 else 0):
                # depth-2 software pipeline: softmax chain for ec+1 is
                # emitted before the matmuls of ec
                if ec == 0 and ec not in a_trs:
                    a_trs[ec] = p2_softmax(ec)
                if ec + 1 < ND and (ec + 1) not in a_trs:
                    a_trs[ec + 1] = p2_softmax(ec + 1)
                a_tr, rz2 = a_trs.pop(ec)
                p2_matmul(ec, a_tr, rz2)

    nc.compile()
    return nc


def get_nc(spp):
    if spp not in _CACHED:
        _CACHED[spp] = _build(spp)
    return _CACHED[spp]


def _make_in_maps(state, x, src):
    # Host-side prep: gather kept rows (src != 0), cast to fp16, pad to a
    # multiple of 128, and pre-transpose both state and x. fp16 conversion
    # happens host-side: the device would round both operands to fp16 before
    # the matmuls anyway (same numerics), and this halves input DMA bytes.
    state = np.asarray(state, dtype=np.float16)
    x = np.asarray(x, dtype=np.float16)
    src = np.asarray(src)
    idxs = [np.flatnonzero(src[b] != 0) for b in range(B)]
    smax = max(len(i) for i in idxs)
    spp = max(128, ((smax + 127) // 128) * 128)
    in_maps = []
    for b in range(B):
        idx = idxs[b]
        n = len(idx)
        sg = np.zeros((spp, D), np.float16)
        sg[:n] = state[b][idx]
        xg = np.zeros((spp, D), np.float16)
        xg[:n] = x[b][idx]
        keep = np.zeros(spp, np.float16)
        keep[:n] = 1.0
        in_maps.append(
            {
                "state": sg,
                "state_t": np.ascontiguousarray(sg.T),
                "x_t": np.ascontiguousarray(xg.T),
                "keep": keep,
            }
        )
    return in_maps, spp


def run_bass(state, x, src, trace=False, **trace_kwargs):
    from concourse.bass_utils import run_bass_kernel_spmd

    in_maps, spp = _make_in_maps(state, x, src)
    nc = get_nc(spp)
    res = run_bass_kernel_spmd(
        nc, in_maps, core_ids=list(range(B)), trace=trace, **trace_kwargs
    )
    out = np.stack([res.results[b]["out"] for b in range(B)]).astype(np.float32)
    return out, res


def kernel(state, x, src, **kwargs):
    out, _ = run_bass(state, x, src, trace=False)
    return out


if __name__ == "__main__":
    rng = np.random.default_rng(0)
    st = rng.standard_normal((B, S, D), dtype=np.float32)
    xx = rng.standard_normal((B, S, D), dtype=np.float32)
    sr = rng.integers(0, 5, size=(B, S))
    o = kernel(state=st, x=xx, src=sr)
    print(o.shape, o.dtype, np.abs(o).max())
